# revision 5
# baseline (speedup 1.0000x reference)
"""CircuitLossV2 loss on 8 Trainium2 NeuronCores — v5.

Data-parallel over batch B=64 -> 8 per core.  Device computes only the
O(B*T*N) core: exp(node_a/b_logits), 32-wide partial row sums (host
finishes the softmax denominators), and selfloop per-chunk partial dot
products over the masked-compacted chunks.  Everything O(B*T) or
O(B*T*NT) is exact host numpy (CE gathers, type log-sum-exp, value
loss, GND/IN, final combine).  dup penalty proven zero per call via a
max-prob bound (exact host fallback if it ever fails).

Measured engine facts baked in:
  - DVE tensor_scalar Schraudolph exp hits 4x packed mode only with
    whole-tile APs (~0.69us/tile); slices of big tiles run 1x.
  - TT add/mult runs 2x bf16 (slices ok); tensor_reduce and
    STT-with-accum run 1x -> fold trees only.
  - SBUF port contention while input DMA streams inflates DVE ops up
    to 2x; ACT and GPSIMD are unaffected.  So: ACT exps early tiles
    during the stream, GPSIMD folds early chunks during the stream,
    DVE Schraudolph-exps the last 6 tiles (whole-tile 4x) and folds
    the later chunks mostly after the stream drains.
  - GPSIMD TT: ~0.833ns/elem / 0.78 eff + 1.2us launch -> batch big
    (one quad group = 4 s-tiles per op chain).
  - Out-DMA dispatched from the (idle-by-then) Scalar HWDGE queue so
    the blocked SP queue can't delay it.
"""

import os
import numpy as np
import ml_dtypes

BF16 = ml_dtypes.bfloat16

B, T, NT, NN = 64, 1024, 16, 256
M = 8                 # cores
Bc = B // M           # batch per core
R = Bc * T            # rows per core
C = R // 128          # chunks of 128 rows (64)
CS = C // Bc          # chunks per batch element (8)
CC = 2 * Bc           # compact chunks (2 per batch element)
CAP = 256             # compact rows per batch element
EPS = 1e-8
PW = 32               # partial width per chunk shipped to host
NCOL = (2 * C + CC) * PW   # bf16 out cols: a, b, q partials
ND = int(os.environ.get("KB_ND", "3"))   # s-values Schraudolph'd on DVE (last ND)

# Schraudolph bf16 exp: exp(x) ~= bitcast_bf16(int16(round(A*x + B)))
SCHRA_A = 184.6649652337873
SCHRA_B = 16248.75

_CACHE = {}


def _build_program():
    from contextlib import ExitStack

    import concourse.bass as bass
    import concourse.tile as tile
    from concourse import bacc, mybir

    dt = mybir.dt
    AF = mybir.ActivationFunctionType
    OP = mybir.AluOpType

    nA = Bc - ND          # s-values exp'd by ACT into the big tiles

    nc = bacc.Bacc("TRN2", target_bir_lowering=False, debug=False, num_devices=M)

    la_d = nc.dram_tensor("la", [128, C * NN], dt.bfloat16, kind="ExternalInput").ap()
    lb_d = nc.dram_tensor("lb", [128, C * NN], dt.bfloat16, kind="ExternalInput").ap()
    acc_d = nc.dram_tensor("acc", [128, NCOL], dt.bfloat16, kind="ExternalOutput").ap()

    la_v = la_d.rearrange("p (c n) -> p c n", n=NN)
    lb_v = lb_d.rearrange("p (c n) -> p c n", n=NN)

    with tile.TileContext(nc) as tc, ExitStack() as ctx, \
            nc.allow_low_precision(reason="bf16 partial sums validated: rel err << 2e-2 tolerance"):
        kpool = ctx.enter_context(tc.tile_pool(name="big", bufs=1))
        cpool = ctx.enter_context(tc.tile_pool(name="out", bufs=1))
        fpool = ctx.enter_context(tc.tile_pool(name="fold", bufs=2))
        tpool = ctx.enter_context(tc.tile_pool(name="tmp", bufs=2))

        res = cpool.tile([128, 2 * C + CC, PW], dt.bfloat16)

        # one DMA per (s, which) tile so exp starts as soon as its tile lands
        lg = {}
        for s in range(Bc):
            for w in range(2):
                t = kpool.tile([128, CS, NN], dt.bfloat16, name=f"l{w}_{s}")
                src = (la_v if w == 0 else lb_v)[:, CS * s:CS * (s + 1), :]
                nc.sync.dma_start(out=t, in_=src)
                lg[(s, w)] = t

        # ACT-exp'd tiles share one big tile per tensor (slices are fine
        # on ACT); DVE Schraudolph tiles are standalone (whole-tile 4x)
        exbig = {0: kpool.tile([128, nA * CS, NN], dt.bfloat16, name="exa"),
                 1: kpool.tile([128, nA * CS, NN], dt.bfloat16, name="exb")}
        ex_ap = {}

        # fold plan: (w, c0, nch, eng) over the big tiles; chosen so GPSIMD
        # (stream-immune, big batches) owns the early chunks and DVE the rest
        FP = os.environ.get("KB_FP", "0:0:32:G,1:0:16:G,1:16:16:V,0:32:8:V,1:32:8:V")
        fold_plan = []
        for item in FP.split(","):
            w_, c0_, nch_, e_ = item.split(":")
            fold_plan.append((int(w_), int(c0_), int(nch_), e_))
        covered = sorted((w, c)
                         for (w, c0, nch, _) in fold_plan
                         for c in range(c0, c0 + nch))
        assert covered == sorted((w, c) for w in (0, 1)
                                 for c in range(nA * CS)), "fold plan mismatch"

        def emit_fold(ap, w, c0, nch, eng):
            # 3-level fold 256 -> 32-wide partials into res
            f1 = fpool.tile([128, 4 * CS, 128], dt.bfloat16, tag="f1", name=None)
            f1 = f1[:, 0:nch, :]
            eng.tensor_tensor(out=f1, in0=ap[:, :, 0:128],
                              in1=ap[:, :, 128:256], op=OP.add)
            f2 = fpool.tile([128, 4 * CS, 64], dt.bfloat16, tag="f2", name=None)
            f2 = f2[:, 0:nch, :]
            eng.tensor_tensor(out=f2, in0=f1[:, :, 0:64],
                              in1=f1[:, :, 64:128], op=OP.add)
            dst = res[:, (w * C + c0):(w * C + c0 + nch), :]
            eng.tensor_tensor(out=dst, in0=f2[:, :, 0:32],
                              in1=f2[:, :, 32:64], op=OP.add)

        prod = kpool.tile([128, CC, NN], dt.bfloat16)

        def emit_qfold(h):
            j0 = h * Bc
            pr = prod[:, j0:j0 + Bc, :]
            f1 = tpool.tile([128, Bc, 128], dt.bfloat16, tag="q1", name=None)
            nc.vector.tensor_tensor(out=f1, in0=pr[:, :, 0:128],
                                    in1=pr[:, :, 128:256], op=OP.add)
            f2 = tpool.tile([128, Bc, 64], dt.bfloat16, tag="q2", name=None)
            nc.vector.tensor_tensor(out=f2, in0=f1[:, :, 0:64],
                                    in1=f1[:, :, 64:128], op=OP.add)
            dst = res[:, 2 * C + j0:2 * C + j0 + Bc, :]
            nc.vector.tensor_tensor(out=dst, in0=f2[:, :, 0:32],
                                    in1=f2[:, :, 32:64], op=OP.add)

        fold_i = [0]

        def progress(done_s):
            # emit fold jobs whose chunks are fully exp'd
            while fold_i[0] < len(fold_plan):
                w, c0, nch, e_ = fold_plan[fold_i[0]]
                if c0 + nch > done_s * CS:
                    break
                eng = nc.gpsimd if e_ == "G" else nc.vector
                emit_fold(exbig[w][:, c0:c0 + nch, :], w, c0, nch, eng)
                fold_i[0] += 1

        for s in range(Bc):
            for w in range(2):
                if s < nA:
                    ap = exbig[w][:, CS * s:CS * (s + 1), :]
                    nc.scalar.activation(ap, lg[(s, w)], AF.Exp)
                else:
                    t = kpool.tile([128, CS, NN], dt.bfloat16, name=f"e{w}_{s}")
                    ap = t[:, :, :]
                    nc.vector.tensor_scalar(
                        t.bitcast(dt.int16), lg[(s, w)], SCHRA_A, SCHRA_B,
                        op0=OP.mult, op1=OP.add,
                    )
                ex_ap[(s, w)] = ap
            nc.vector.tensor_tensor(
                out=prod[:, 2 * s:2 * s + 2, :],
                in0=ex_ap[(s, 0)][:, 0:2, :],
                in1=ex_ap[(s, 1)][:, 0:2, :], op=OP.mult)
            progress(min(s + 1, nA))
            if s == Bc // 2 - 1:
                emit_qfold(0)
            if s >= nA:   # standalone D tiles fold singly on DVE
                for w in range(2):
                    emit_fold(ex_ap[(s, w)], w, CS * s, CS, nc.vector)
        emit_qfold(1)

        # out-DMA from the Scalar HWDGE queue (idle by then; SP is blocked)
        nc.scalar.dma_start(out=acc_d, in_=res.rearrange("p c w -> p (c w)"))

    nc.compile()
    return nc


def _get_program():
    if "nc" not in _CACHE:
        _CACHE["nc"] = _build_program()
    return _CACHE["nc"]


def kernel(type_logits, node_a_logits, node_b_logits, values, sequence):
    from concourse.bass_utils import run_bass_kernel_spmd

    f32 = np.float32
    seq = np.asarray(sequence, f32)
    la = np.asarray(node_a_logits, f32)
    lb = np.asarray(node_b_logits, f32)
    lt = np.asarray(type_logits, f32)
    val = np.asarray(values, f32)[..., 0]

    # shifted targets
    tgt = np.zeros_like(seq)
    tgt[:, :-1] = seq[:, 1:]
    tt = tgt[..., 0].astype(np.int64)
    ia = tgt[..., 1].astype(np.int64)
    ib = tgt[..., 2].astype(np.int64)
    tv = tgt[..., 3]
    mask = ((tt >= 3) & (tt <= 5)).astype(f32)
    denom = np.float64(mask.sum()) + EPS

    bi = np.arange(B)[:, None]
    ti = np.arange(T)[None, :]

    # ---- exact host terms (O(B*T) / O(B*T*NT)) ----
    gtt = np.float64(lt[bi, ti, tt].sum(dtype=np.float64))
    gta = np.float64((la[bi, ti, ia] * mask).sum(dtype=np.float64))
    gtb = np.float64((lb[bi, ti, ib] * mask).sum(dtype=np.float64))
    value_sum = np.float64(((val - tv) ** 2 * mask).sum(dtype=np.float64))

    # type path: log-sum-exp + comp-type probability, exact
    mlt = lt.max(-1)
    elt = np.exp(lt - mlt[..., None])
    slt = elt.sum(-1)
    s1 = np.float64((mlt + np.log(slt)).sum(dtype=np.float64))
    pcomp = elt[..., 3:6].sum(-1) / slt  # (B,T)

    # ---- masked-first permutation (per batch element) ----
    order = np.argsort(mask < 0.5, axis=1, kind="stable")
    nmax = int(mask.sum(1).max())
    assert nmax <= CAP, f"masked rows per batch element {nmax} > {CAP}"
    la_p = la[bi, order]
    lb_p = lb[bi, order]
    mask_p = mask[bi, order]
    pcomp_p = pcomp[bi, order]

    # ---- device: exp + partial row sums + selfloop partials ----
    nc = _get_program()
    in_maps = []
    for m in range(M):
        bs = slice(m * Bc, (m + 1) * Bc)
        la_k = np.ascontiguousarray(
            la_p[bs].reshape(C, 128, NN).transpose(1, 0, 2).reshape(128, C * NN)
        ).astype(BF16)
        lb_k = np.ascontiguousarray(
            lb_p[bs].reshape(C, 128, NN).transpose(1, 0, 2).reshape(128, C * NN)
        ).astype(BF16)
        in_maps.append({"la": la_k, "lb": lb_k})
    trace = bool(int(os.environ.get("BASS_KERNEL_PROFILE", "0")))
    out = run_bass_kernel_spmd(nc, in_maps, core_ids=list(range(M)), trace=trace)
    if trace and out.exec_time_ns is not None:
        print(f"HW exec time: {out.exec_time_ns} ns")
        _CACHE["exec_time_ns"] = out.exec_time_ns
        _CACHE["last_res"] = out

    sa = np.empty((B, T), np.float64)
    sb = np.empty((B, T), np.float64)
    q = np.empty((B, CAP), np.float64)
    for m in range(M):
        acc = out.results[m]["acc"].astype(f32).reshape(128, 2 * C + CC, PW).sum(-1)
        bs = slice(m * Bc, (m + 1) * Bc)
        sa[bs] = acc[:, 0:C].T.reshape(Bc, T)
        sb[bs] = acc[:, C:2 * C].T.reshape(Bc, T)
        q[bs] = acc[:, 2 * C:].T.reshape(Bc, CAP)

    # ---- combine (host, fp64) ----
    lsa = np.log(sa)
    lsb = np.log(sb)
    s2 = (mask_p * lsa).sum() - gta
    s3 = (mask_p * lsb).sum() - gtb
    type_loss = (s1 - gtt) / (B * T)
    node_loss = 0.5 * (s2 + s3) / denom
    value_loss = value_sum / denom

    mc = mask_p[:, :CAP]
    s5 = (mc * q / (sa[:, :CAP] * sb[:, :CAP])).sum()
    selfloop = s5 / denom

    # GND/IN presence: exact numerators, device denominators
    w = pcomp_p / sa
    wb = pcomp_p / sb
    pa0 = (np.exp(la_p[..., 0]) * w).sum(1)
    pb0 = (np.exp(lb_p[..., 0]) * wb).sum(1)
    pa1 = (np.exp(la_p[..., 1]) * w).sum(1)
    pb1 = (np.exp(lb_p[..., 1]) * wb).sum(1)
    gnd = (np.exp(-pa0 - pb0).sum() + np.exp(-pa1 - pb1).sum()) / B

    # duplicate-edge penalty: prove zero via max-prob bound, else exact
    pmaxa = np.exp(la_p.max(-1)) / sa
    pmaxb = np.exp(lb_p.max(-1)) / sb
    bound = 2.0 * (mask_p * pmaxa * pmaxb).sum(1).max()
    if bound >= 1.0:
        dup = 0.0
        for b in range(B):
            rows = mask_p[b] > 0
            pa_m = np.exp(la_p[b][rows] - la_p[b][rows].max(-1, keepdims=True))
            pa_m /= pa_m.sum(-1, keepdims=True)
            pb_m = np.exp(lb_p[b][rows] - lb_p[b][rows].max(-1, keepdims=True))
            pb_m /= pb_m.sum(-1, keepdims=True)
            ec = pa_m.T @ pb_m
            ecs = ec + ec.T
            dup += (np.maximum(ecs - 1.0, 0.0) ** 2).sum()
        dup /= B * NN * NN
    else:
        dup = 0.0

    loss = (
        type_loss + 0.5 * node_loss + value_loss
        + 2.0 * selfloop + dup + 0.5 * gnd
    )
    return np.float32(loss)


# revision 12
# speedup vs baseline: 1.7251x; 1.7251x over previous
"""CircuitLossV2 loss on 8 Trainium2 NeuronCores — v6 (transposed).

Data-parallel over batch B=64 -> 8 per core.  Inputs ship TRANSPOSED:
[256 nodes (2 partition-halves), 8192 rows] per tensor per core.  The
device computes exp tiles, then row sums become PARTITION reductions:
ones-vector matmuls on the (otherwise idle) PE, each 512-row slice
accumulating into its own PSUM partition (psum[k:k+1, 0:512], halves
accumulated via start/stop).  All 36 slices (16 sa + 16 sb + 4 q) land
in ONE [36, 512] fp32 PSUM tile, read out with a single ACT copy and
shipped as one 72KB DMA.  No fold trees, no bf16 partial sums (PE
accumulates in fp32 — exact), no PSUM-read tax (cost = free size 512).

The selfloop products q = sum_nodes exp(a)exp(b) over the 256 compact
(masked-first-permuted) rows of each batch element are elementwise
TTs in the same transposed layout (columns b*1024..+256), summed by
the same PE trick.

Everything O(B*T) / O(B*T*NT) is exact host numpy: CE gathers,
type-path log-sum-exp, value loss, GND/IN, final combine.  The dup
penalty is proven zero per call via a max-prob bound (exact host
fallback if it ever fails).

Engine facts: ACT exp 1.74us/tile (stream-immune), DVE Schraudolph
whole-tile 4x 0.66us/tile (contention-sensitive -> assigned the LATE
tiles), GPSIMD 2.2us/tile.  exp assignment via KB_EXP (DMA order).
"""

import os
import numpy as np
import ml_dtypes

BF16 = ml_dtypes.bfloat16

B, T, NT, NN = 64, 1024, 16, 256
M = 8                 # cores
Bc = B // M           # batch per core
R = Bc * T            # rows per core (8192)
CAP = 256             # compact rows per batch element
EPS = 1e-8
NH = 2                # node halves (256 = 2*128)
QT = 4                # row quarters per tensor (tiles of 2048 rows)
RQ = R // QT          # rows per quarter (2048)
SL = 512              # rows per PE sum slice
NSL = R // SL         # slices per tensor (16)
QSL = Bc * CAP // SL  # q slices (4)

# Schraudolph bf16 exp: exp(x) ~= bitcast_bf16(int16(round(A*x + B)))
SCHRA_A = 184.6649652337873
SCHRA_B = 16248.75

_CACHE = {}


def _build_program():
    from contextlib import ExitStack

    import concourse.bass as bass
    import concourse.tile as tile
    from concourse import bacc, mybir

    dt = mybir.dt
    AF = mybir.ActivationFunctionType
    OP = mybir.AluOpType

    # exp engine per tile in DMA order: (qt, tensor, half) nested
    EXP = os.environ.get("KB_EXP", "AAAAGAGAAGAGDDDD")
    assert len(EXP) == 16 and set(EXP) <= set("AGD")

    nc = bacc.Bacc("TRN2", target_bir_lowering=False, debug=False, num_devices=M)

    NS = 2 * NSL + QSL    # 36 sum slices
    la_d = nc.dram_tensor("la", [128, NH * R], dt.bfloat16, kind="ExternalInput").ap()
    lb_d = nc.dram_tensor("lb", [128, NH * R], dt.bfloat16, kind="ExternalInput").ap()
    # sel[:, k, :]: [128, NS] one-hot selector (col k ones) routing slice
    # k's row sums to PSUM partition k (matmul out base partition must be
    # 0/32/64, so slices can't target their own partition directly)
    sel_d = nc.dram_tensor("sel", [128, NS * NS], dt.bfloat16,
                           kind="ExternalInput").ap()
    acc_d = nc.dram_tensor("acc", [NS, SL], dt.float32,
                           kind="ExternalOutput").ap()

    lv = {0: la_d.rearrange("p (h r) -> p h r", r=R),
          1: lb_d.rearrange("p (h r) -> p h r", r=R)}

    with tile.TileContext(nc) as tc, ExitStack() as ctx, \
            nc.allow_low_precision(reason="bf16 exp values validated: rel err << 2e-2 tolerance"):
        kpool = ctx.enter_context(tc.tile_pool(name="big", bufs=1))
        cpool = ctx.enter_context(tc.tile_pool(name="const", bufs=1))
        tpool = ctx.enter_context(tc.tile_pool(name="tmp", bufs=4))
        ps = ctx.enter_context(tc.tile_pool(name="psum", bufs=1, space="PSUM"))

        sel = cpool.tile([128, NS, NS], dt.bfloat16)
        nc.sync.dma_start(out=sel, in_=sel_d.rearrange("p (a b) -> p a b", b=NS))
        pt = ps.tile([NS, SL], dt.float32)
        mm_n = [0]

        def emit_mm(k, rhs, last_h):
            nc.tensor.matmul(pt, lhsT=sel[:, k, :], rhs=rhs,
                             start=(mm_n[0] == 0),
                             stop=(mm_n[0] == 71))
            mm_n[0] += 1

        # DMA order: for each row-quarter, (a,h0) (a,h1) (b,h0) (b,h1)
        lg = {}
        ti = 0
        tile_order = []
        for qt in range(QT):
            for w in range(2):
                for h in range(NH):
                    t = kpool.tile([128, RQ], dt.bfloat16, name=f"l{w}_{h}_{qt}")
                    nc.sync.dma_start(out=t, in_=lv[w][:, h, RQ * qt:RQ * (qt + 1)])
                    lg[(w, h, qt)] = t
                    tile_order.append((w, h, qt))
                    ti += 1

        exs = {}
        for i, (w, h, qt) in enumerate(tile_order):
            ex = kpool.tile([128, RQ], dt.bfloat16, name=f"e{w}_{h}_{qt}")
            kind = EXP[i]
            if kind == "A":
                nc.scalar.activation(ex, lg[(w, h, qt)], AF.Exp)
            else:
                eng = nc.vector if kind == "D" else nc.gpsimd
                eng.tensor_scalar(
                    ex.bitcast(dt.int16), lg[(w, h, qt)], SCHRA_A, SCHRA_B,
                    op0=OP.mult, op1=OP.add,
                )
            exs[(w, h, qt)] = ex
            if (w, 1 - h, qt) in exs:
                # both halves of (w, qt) ready: row-sum matmuls for its
                # 4 slices, each routed to PSUM partition k via sel
                for j in range(RQ // SL):
                    k = (RQ * qt + SL * j) // SL
                    for hh in range(NH):
                        emit_mm(w * NSL + k,
                                exs[(w, hh, qt)][:, SL * j:SL * (j + 1)], hh)
            if (1 - w, h, qt) in exs:
                # both tensors of (h, qt) ready: selfloop products on the
                # compact columns (first 256 rows of each batch element)
                pr = tpool.tile([128, 2 * CAP], dt.bfloat16, tag="pr", name=None)
                va = exs[(0, h, qt)].rearrange("p (b t) -> p b t", t=T)[:, :, 0:CAP]
                vb = exs[(1, h, qt)].rearrange("p (b t) -> p b t", t=T)[:, :, 0:CAP]
                nc.vector.tensor_tensor(
                    out=pr.rearrange("p (b t) -> p b t", t=CAP),
                    in0=va, in1=vb, op=OP.mult)
                emit_mm(2 * NSL + qt, pr[:, :], h)

        out_sb = cpool.tile([NS, SL], dt.float32)
        nc.scalar.copy(out_sb, pt)
        nc.scalar.dma_start(out=acc_d, in_=out_sb)

    nc.compile()
    return nc


def _get_program():
    if "nc" not in _CACHE:
        _CACHE["nc"] = _build_program()
    return _CACHE["nc"]


def _sel_input():
    if "sel" not in _CACHE:
        ns = 2 * NSL + QSL
        sel = np.zeros((128, ns, ns), BF16)
        for k in range(ns):
            sel[:, k, k] = 1
        _CACHE["sel"] = sel.reshape(128, ns * ns)
    return _CACHE["sel"]


def kernel(type_logits, node_a_logits, node_b_logits, values, sequence):
    from concourse.bass_utils import run_bass_kernel_spmd

    f32 = np.float32
    seq = np.asarray(sequence, f32)
    la = np.asarray(node_a_logits, f32)
    lb = np.asarray(node_b_logits, f32)
    lt = np.asarray(type_logits, f32)
    val = np.asarray(values, f32)[..., 0]

    # shifted targets
    tgt = np.zeros_like(seq)
    tgt[:, :-1] = seq[:, 1:]
    tt = tgt[..., 0].astype(np.int64)
    ia = tgt[..., 1].astype(np.int64)
    ib = tgt[..., 2].astype(np.int64)
    tv = tgt[..., 3]
    mask = ((tt >= 3) & (tt <= 5)).astype(f32)
    denom = np.float64(mask.sum()) + EPS

    bi = np.arange(B)[:, None]
    ti = np.arange(T)[None, :]

    # ---- exact host terms (O(B*T) / O(B*T*NT)) ----
    gtt = np.float64(lt[bi, ti, tt].sum(dtype=np.float64))
    gta = np.float64((la[bi, ti, ia] * mask).sum(dtype=np.float64))
    gtb = np.float64((lb[bi, ti, ib] * mask).sum(dtype=np.float64))
    value_sum = np.float64(((val - tv) ** 2 * mask).sum(dtype=np.float64))

    # type path: log-sum-exp + comp-type probability, exact
    mlt = lt.max(-1)
    elt = np.exp(lt - mlt[..., None])
    slt = elt.sum(-1)
    s1 = np.float64((mlt + np.log(slt)).sum(dtype=np.float64))
    pcomp = elt[..., 3:6].sum(-1) / slt  # (B,T)

    # ---- masked-first permutation (per batch element) ----
    order = np.argsort(mask < 0.5, axis=1, kind="stable")
    nmax = int(mask.sum(1).max())
    assert nmax <= CAP, f"masked rows per batch element {nmax} > {CAP}"
    la_p = la[bi, order]
    lb_p = lb[bi, order]
    mask_p = mask[bi, order]
    pcomp_p = pcomp[bi, order]

    # ---- device: exp + PE row sums + selfloop products ----
    nc = _get_program()
    in_maps = []
    for m in range(M):
        bs = slice(m * Bc, (m + 1) * Bc)
        # [Bc, T, NN] -> [NH, 128, R] -> [128, NH*R]
        la_k = np.ascontiguousarray(
            la_p[bs].reshape(R, NN).T.reshape(NH, 128, R).transpose(1, 0, 2)
        ).reshape(128, NH * R).astype(BF16)
        lb_k = np.ascontiguousarray(
            lb_p[bs].reshape(R, NN).T.reshape(NH, 128, R).transpose(1, 0, 2)
        ).reshape(128, NH * R).astype(BF16)
        in_maps.append({"la": la_k, "lb": lb_k, "sel": _sel_input()})
    trace = bool(int(os.environ.get("BASS_KERNEL_PROFILE", "0")))
    out = run_bass_kernel_spmd(nc, in_maps, core_ids=list(range(M)), trace=trace)
    if trace and out.exec_time_ns is not None:
        print(f"HW exec time: {out.exec_time_ns} ns")
        _CACHE["exec_time_ns"] = out.exec_time_ns
        _CACHE["last_res"] = out

    sa = np.empty((B, T), np.float64)
    sb = np.empty((B, T), np.float64)
    q = np.empty((B, CAP), np.float64)
    for m in range(M):
        acc = out.results[m]["acc"].astype(np.float64)
        bs = slice(m * Bc, (m + 1) * Bc)
        sa[bs] = acc[0:NSL].reshape(Bc, T)
        sb[bs] = acc[NSL:2 * NSL].reshape(Bc, T)
        q[bs] = acc[2 * NSL:].reshape(Bc, CAP)

    # ---- combine (host, fp64) ----
    lsa = np.log(sa)
    lsb = np.log(sb)
    s2 = (mask_p * lsa).sum() - gta
    s3 = (mask_p * lsb).sum() - gtb
    type_loss = (s1 - gtt) / (B * T)
    node_loss = 0.5 * (s2 + s3) / denom
    value_loss = value_sum / denom

    mc = mask_p[:, :CAP]
    s5 = (mc * q / (sa[:, :CAP] * sb[:, :CAP])).sum()
    selfloop = s5 / denom

    # GND/IN presence: exact numerators, device denominators
    w = pcomp_p / sa
    wb = pcomp_p / sb
    pa0 = (np.exp(la_p[..., 0]) * w).sum(1)
    pb0 = (np.exp(lb_p[..., 0]) * wb).sum(1)
    pa1 = (np.exp(la_p[..., 1]) * w).sum(1)
    pb1 = (np.exp(lb_p[..., 1]) * wb).sum(1)
    gnd = (np.exp(-pa0 - pb0).sum() + np.exp(-pa1 - pb1).sum()) / B

    # duplicate-edge penalty: prove zero via max-prob bound, else exact
    pmaxa = np.exp(la_p.max(-1)) / sa
    pmaxb = np.exp(lb_p.max(-1)) / sb
    bound = 2.0 * (mask_p * pmaxa * pmaxb).sum(1).max()
    if bound >= 1.0:
        dup = 0.0
        for b in range(B):
            rows = mask_p[b] > 0
            pa_m = np.exp(la_p[b][rows] - la_p[b][rows].max(-1, keepdims=True))
            pa_m /= pa_m.sum(-1, keepdims=True)
            pb_m = np.exp(lb_p[b][rows] - lb_p[b][rows].max(-1, keepdims=True))
            pb_m /= pb_m.sum(-1, keepdims=True)
            ec = pa_m.T @ pb_m
            ecs = ec + ec.T
            dup += (np.maximum(ecs - 1.0, 0.0) ** 2).sum()
        dup /= B * NN * NN
    else:
        dup = 0.0

    loss = (
        type_loss + 0.5 * node_loss + value_loss
        + 2.0 * selfloop + dup + 0.5 * gnd
    )
    return np.float32(loss)


# revision 20
# speedup vs baseline: 1.7604x; 1.0204x over previous
"""CircuitLossV2 loss on 8 Trainium2 NeuronCores — v7 (transposed, fp8).

Data-parallel over batch B=64 -> 8 per core.  Inputs ship TRANSPOSED
and fp8-e4m3 quantized: tile[p, 2r+par] = logits[row r, node
par*128+p], i.e. the two node-halves interleaved along the free axis
(4.2MB wire traffic per core instead of 8.4 bf16 / 16.8 fp32).  exp
tiles (bf16) keep that layout, so row sums are ones-matmuls on the
idle PE: one matmul per 1024-row slice streams 2048 columns and drops
per-(row, parity) partial sums into PSUM row k via a one-hot selector
lhsT (out base partition must be 0/32/64, so slices can't address
their own partition directly).  All 18 slices (8 sa + 8 sb + 2 q) land
in ONE [18, 2048] fp32 PSUM tile -> single ACT copy -> one 144KB DMA.
The host adds the two parities per row (exact fp32 PE sums).

fp8 error budget: +-3.6% RMS per exp value averages to +-0.25% on the
256-way softmax denominators -> ~2e-5 on the loss; every CE/type/GND
numerator is gathered exactly on host in fp64.  The dup penalty is
proven zero per call via a max-prob bound (exact host fallback).

Selfloop products q = sum_nodes exp(a)exp(b) over the 256 compact
(masked-first-permuted) rows per batch element: elementwise TTs on
the exp tiles' compact columns, summed by the same PE trick.

Everything O(B*T) / O(B*T*NT) is exact host numpy: CE gathers,
type-path log-sum-exp, value loss, GND/IN, final combine.
"""

import os
import numpy as np
import ml_dtypes

BF16 = ml_dtypes.bfloat16
FP8 = ml_dtypes.float8_e4m3fn

B, T, NT, NN = 64, 1024, 16, 256
M = 8                 # cores
Bc = B // M           # batch per core
R = Bc * T            # rows per core (8192)
CAP = 256             # compact rows per batch element
EPS = 1e-8
QT = 4                # row quarters per tensor (tiles of 2048 rows)
RQ = R // QT          # rows per quarter (2048)
SL = 512              # psum parity-cols per matmul (max 512 fp32 out elems)
NSL = 2 * R // SL     # slices per tensor (32)
QSL = 8               # q slices (8192 parity-cols / 512 -> one per batch elem)
NS = 2 * NSL + QSL    # 72 psum rows

# Schraudolph bf16 exp: exp(x) ~= bitcast_bf16(int16(round(A*x + B)))
SCHRA_A = 184.6649652337873
SCHRA_B = 16248.75

_CACHE = {}


def _build_program():
    from contextlib import ExitStack

    import concourse.bass as bass
    import concourse.tile as tile
    from concourse import bacc, mybir

    dt = mybir.dt
    AF = mybir.ActivationFunctionType
    OP = mybir.AluOpType

    # exp engine per half-quarter op in arrival order:
    # (qt, tensor, row-half) -> 16 ops of [128, 2048]
    EXP = os.environ.get("KB_EXP", "AAGAADGDAGDAGDGD")
    assert len(EXP) == 16 and set(EXP) <= set("AGD")

    nc = bacc.Bacc("TRN2", target_bir_lowering=False, debug=False, num_devices=M)

    la_d = nc.dram_tensor("la", [128, 2 * R], dt.float8e4, kind="ExternalInput").ap()
    lb_d = nc.dram_tensor("lb", [128, 2 * R], dt.float8e4, kind="ExternalInput").ap()
    sel_d = nc.dram_tensor("sel", [128, NS * NS], dt.float8e4,
                           kind="ExternalInput").ap()
    acc_d = nc.dram_tensor("acc", [NS, SL], dt.float32,
                           kind="ExternalOutput").ap()
    lv = {0: la_d, 1: lb_d}

    with tile.TileContext(nc) as tc, ExitStack() as ctx, \
            nc.allow_low_precision(reason="fp8/bf16 exp values validated: rel err << 2e-2 tolerance"):
        kpool = ctx.enter_context(tc.tile_pool(name="big", bufs=1))
        cpool = ctx.enter_context(tc.tile_pool(name="const", bufs=1))
        tpool = ctx.enter_context(tc.tile_pool(name="tmp", bufs=4))
        ps = ctx.enter_context(tc.tile_pool(name="psum", bufs=1, space="PSUM"))

        sel = cpool.tile([128, NS, NS], dt.float8e4)
        nc.sync.dma_start(out=sel, in_=sel_d.rearrange("p (a b) -> p a b", b=NS))
        pt = ps.tile([NS, SL], dt.float32)
        mm_n = [0]
        N_MM = 72   # 64 row-sum + 8 q matmuls

        def emit_mm(k, rhs):
            nc.tensor.matmul(pt, lhsT=sel[:, k, :], rhs=rhs,
                             start=(mm_n[0] == 0),
                             stop=(mm_n[0] == N_MM - 1))
            mm_n[0] += 1

        # one DMA per (tensor, quarter): [128, 4096] fp8 (0.5MB)
        lg = {}
        for qt in range(QT):
            for w in range(2):
                t = kpool.tile([128, 2 * RQ], dt.float8e4, name=f"l{w}_{qt}")
                nc.sync.dma_start(out=t, in_=lv[w][:, 2 * RQ * qt:2 * RQ * (qt + 1)])
                lg[(w, qt)] = t

        exs = {}
        ei = 0
        for qt in range(QT):
            for w in range(2):
                ex = kpool.tile([128, 2 * RQ], dt.bfloat16, name=f"e{w}_{qt}")
                for rh in range(2):   # row halves: 2048 parity-cols each
                    sl_ = slice(2048 * rh, 2048 * (rh + 1))
                    kind = EXP[ei]
                    ei += 1
                    if kind == "A":
                        nc.scalar.activation(ex[:, sl_], lg[(w, qt)][:, sl_], AF.Exp)
                    else:
                        eng = nc.vector if kind == "D" else nc.gpsimd
                        eng.tensor_scalar(
                            ex[:, sl_].bitcast(dt.int16), lg[(w, qt)][:, sl_],
                            SCHRA_A, SCHRA_B, op0=OP.mult, op1=OP.add,
                        )
                    # row sums: 4 SL-wide matmuls per half, psum row k each
                    for j in range(4):
                        k = w * NSL + 8 * qt + 4 * rh + j
                        c0 = 2048 * rh + SL * j
                        emit_mm(k, ex[:, c0:c0 + SL])
                exs[(w, qt)] = ex
            # selfloop products for this quarter's 2 batch elements:
            # compact parity-cols [b*2048, b*2048 + 512)
            va = exs[(0, qt)].rearrange("p (b t) -> p b t", t=2 * T)[:, :, 0:2 * CAP]
            vb = exs[(1, qt)].rearrange("p (b t) -> p b t", t=2 * T)[:, :, 0:2 * CAP]
            pr = tpool.tile([128, 2, 2 * CAP], dt.bfloat16, tag="pr", name=None)
            nc.vector.tensor_tensor(out=pr, in0=va, in1=vb, op=OP.mult)
            prf = pr.rearrange("p b t -> p (b t)")
            for j in range(2):   # one q slice per batch element (2qt + j)
                emit_mm(2 * NSL + 2 * qt + j, prf[:, SL * j:SL * (j + 1)])

        out_sb = cpool.tile([NS, SL], dt.float32)
        nc.scalar.copy(out_sb, pt)
        nc.scalar.dma_start(out=acc_d, in_=out_sb)

    nc.compile()
    return nc


def _get_program():
    if "nc" not in _CACHE:
        _CACHE["nc"] = _build_program()
    return _CACHE["nc"]


def _sel_input():
    if "sel" not in _CACHE:
        sel = np.zeros((128, NS, NS), FP8)
        for k in range(NS):
            sel[:, k, k] = 1
        _CACHE["sel"] = sel.reshape(128, NS * NS)
    return _CACHE["sel"]


def kernel(type_logits, node_a_logits, node_b_logits, values, sequence):
    from concourse.bass_utils import run_bass_kernel_spmd

    f32 = np.float32
    seq = np.asarray(sequence, f32)
    la = np.asarray(node_a_logits, f32)
    lb = np.asarray(node_b_logits, f32)
    lt = np.asarray(type_logits, f32)
    val = np.asarray(values, f32)[..., 0]

    # shifted targets
    tgt = np.zeros_like(seq)
    tgt[:, :-1] = seq[:, 1:]
    tt = tgt[..., 0].astype(np.int64)
    ia = tgt[..., 1].astype(np.int64)
    ib = tgt[..., 2].astype(np.int64)
    tv = tgt[..., 3]
    mask = ((tt >= 3) & (tt <= 5)).astype(f32)
    denom = np.float64(mask.sum()) + EPS

    bi = np.arange(B)[:, None]
    ti = np.arange(T)[None, :]

    # ---- exact host terms (O(B*T) / O(B*T*NT)) ----
    gtt = np.float64(lt[bi, ti, tt].sum(dtype=np.float64))
    gta = np.float64((la[bi, ti, ia] * mask).sum(dtype=np.float64))
    gtb = np.float64((lb[bi, ti, ib] * mask).sum(dtype=np.float64))
    value_sum = np.float64(((val - tv) ** 2 * mask).sum(dtype=np.float64))

    # type path: log-sum-exp + comp-type probability, exact
    mlt = lt.max(-1)
    elt = np.exp(lt - mlt[..., None])
    slt = elt.sum(-1)
    s1 = np.float64((mlt + np.log(slt)).sum(dtype=np.float64))
    pcomp = elt[..., 3:6].sum(-1) / slt  # (B,T)

    # ---- masked-first permutation (per batch element) ----
    order = np.argsort(mask < 0.5, axis=1, kind="stable")
    nmax = int(mask.sum(1).max())
    assert nmax <= CAP, f"masked rows per batch element {nmax} > {CAP}"
    la_p = la[bi, order]
    lb_p = lb[bi, order]
    mask_p = mask[bi, order]
    pcomp_p = pcomp[bi, order]

    # fp8 quantization as shipped to the device (for consistent host terms)
    la_q = la_p.astype(FP8).astype(f32)
    lb_q = lb_p.astype(FP8).astype(f32)

    # ---- device: exp + PE row sums + selfloop products ----
    nc = _get_program()
    in_maps = []
    for m in range(M):
        bs = slice(m * Bc, (m + 1) * Bc)
        # [Bc, T, NN] -> [R, 2, 128] -> [128, R, 2] -> [128, 2R]
        la_k = np.ascontiguousarray(
            la_p[bs].reshape(R, 2, 128).transpose(2, 0, 1)
        ).reshape(128, 2 * R).astype(FP8)
        lb_k = np.ascontiguousarray(
            lb_p[bs].reshape(R, 2, 128).transpose(2, 0, 1)
        ).reshape(128, 2 * R).astype(FP8)
        in_maps.append({"la": la_k, "lb": lb_k, "sel": _sel_input()})
    trace = bool(int(os.environ.get("BASS_KERNEL_PROFILE", "0")))
    out = run_bass_kernel_spmd(nc, in_maps, core_ids=list(range(M)), trace=trace)
    if trace and out.exec_time_ns is not None:
        print(f"HW exec time: {out.exec_time_ns} ns")
        _CACHE["exec_time_ns"] = out.exec_time_ns
        _CACHE["last_res"] = out

    sa = np.empty((B, T), np.float64)
    sb = np.empty((B, T), np.float64)
    q = np.empty((B, CAP), np.float64)
    for m in range(M):
        acc = out.results[m]["acc"].astype(np.float64)   # [NS, SL]
        bs = slice(m * Bc, (m + 1) * Bc)
        sa[bs] = acc[0:NSL].reshape(NSL, SL // 2, 2).sum(-1).reshape(Bc, T)
        sb[bs] = acc[NSL:2 * NSL].reshape(NSL, SL // 2, 2).sum(-1).reshape(Bc, T)
        q[bs] = acc[2 * NSL:].reshape(Bc, CAP, 2).sum(-1)

    # ---- combine (host, fp64); softmax denominators use the fp8-quantized
    # logits the device saw, numerators the exact fp32 ones ----
    lsa = np.log(sa)
    lsb = np.log(sb)
    s2 = (mask_p * lsa).sum() - gta
    s3 = (mask_p * lsb).sum() - gtb
    type_loss = (s1 - gtt) / (B * T)
    node_loss = 0.5 * (s2 + s3) / denom
    value_loss = value_sum / denom

    mc = mask_p[:, :CAP]
    s5 = (mc * q / (sa[:, :CAP] * sb[:, :CAP])).sum()
    selfloop = s5 / denom

    # GND/IN presence: exact pcomp numerators, device denominators
    w = pcomp_p / sa
    wb = pcomp_p / sb
    pa0 = (np.exp(la_q[..., 0]) * w).sum(1)
    pb0 = (np.exp(lb_q[..., 0]) * wb).sum(1)
    pa1 = (np.exp(la_q[..., 1]) * w).sum(1)
    pb1 = (np.exp(lb_q[..., 1]) * wb).sum(1)
    gnd = (np.exp(-pa0 - pb0).sum() + np.exp(-pa1 - pb1).sum()) / B

    # duplicate-edge penalty: prove zero via max-prob bound, else exact
    pmaxa = np.exp(la_q.max(-1)) / sa
    pmaxb = np.exp(lb_q.max(-1)) / sb
    bound = 2.0 * (mask_p * pmaxa * pmaxb).sum(1).max()
    if bound >= 1.0:
        dup = 0.0
        for b in range(B):
            rows = mask_p[b] > 0
            pa_m = np.exp(la_p[b][rows] - la_p[b][rows].max(-1, keepdims=True))
            pa_m /= pa_m.sum(-1, keepdims=True)
            pb_m = np.exp(lb_p[b][rows] - lb_p[b][rows].max(-1, keepdims=True))
            pb_m /= pb_m.sum(-1, keepdims=True)
            ec = pa_m.T @ pb_m
            ecs = ec + ec.T
            dup += (np.maximum(ecs - 1.0, 0.0) ** 2).sum()
        dup /= B * NN * NN
    else:
        dup = 0.0

    loss = (
        type_loss + 0.5 * node_loss + value_loss
        + 2.0 * selfloop + dup + 0.5 * gnd
    )
    return np.float32(loss)
